# revision 46
# baseline (speedup 1.0000x reference)
"""Causal self-attention (QKV proj + RoPE + causal SDPA + out proj) on 8 trn2 cores.

Sharding: tensor-parallel over heads. Each core owns 2 of 16 heads:
  - Wqkv column-split (the core's q/k/v head rows), Wproj row-split.
  - Each core computes a full-shape partial of the output projection;
    the 8 partials are summed (and transposed back) on the host.

Device-side layout trick: everything runs transposed. The host feeds
x^T [C, B*T]; the QKV projection computes qkv^T = Wslice @ x with the
head dim on partitions, which is exactly what Q@K^T and the output
projection want as inputs, so no on-chip transposes are needed except
V (done with DMA xbar transposes, off the critical engines).

Scheduling: panel-level software pipeline.  For global panel g the
emitter weaves three independent instruction streams into the engine
queues so the PE never starves:
  - QKV projection + rope for panel g      (gated on the x DMA)
  - attention for panel g-1                (gated on ACT exp latency)
  - output projection for panel g-2
x panels arrive via single 2MB DMAs on the sync queue, prefetched 3
panels ahead; V transposes are issued from the sync engine; zout
stores go through the gpsimd SWDGE two output-tiles at a time.
"""
import sys

sys.path.insert(0, "/opt/trn_rl_repo")

import numpy as np
import ml_dtypes

import concourse.bacc as bacc
import concourse.mybir as mybir
import concourse.tile as tile
from concourse.bass_utils import run_bass_kernel_spmd

N_CORES = 8
C = 2048
H = 16
D = 128
HPC = H // N_CORES          # heads per core = 2
PB = 512                    # row panel width
JB = 128                    # key tile width
NEG = -1.0e30
ROPE_BASE = 10000.0

BF = mybir.dt.bfloat16
F32 = mybir.dt.float32


def build_module(B, T):
    BT = B * T
    CC = C // 128            # contraction chunks for the projection
    FT = 3 * HPC             # qkv f-tiles per core (q0 q1 k0 k1 v0 v1)
    NPB = T // PB            # panels per batch
    NG = B * NPB             # global panels
    NOC = C // 128           # out-proj column tiles
    NJP = PB // JB           # key blocks per panel
    scale = 1.0 / float(np.sqrt(D))

    nc = bacc.Bacc("TRN2", target_bir_lowering=False, debug=False,
                   num_devices=N_CORES)

    # x pre-tiled on host: xtiles[g, p, cc*PB + r] = x[g*PB + r, cc*128 + p]
    xtiles = nc.dram_tensor("xtiles", [NG, 128, CC * PB], BF,
                            kind="ExternalInput").ap()
    # weights pre-arranged on host so each partition is one contiguous run
    wqkvT = nc.dram_tensor("wqkvT", [128, CC * FT * 128], BF,
                           kind="ExternalInput").ap()
    wprojT = nc.dram_tensor("wprojT", [128, HPC * C], BF,
                            kind="ExternalInput").ap()
    cosT = nc.dram_tensor("cosT", [128, T], BF, kind="ExternalInput").ap()
    sinT = nc.dram_tensor("sinT", [128, T], BF, kind="ExternalInput").ap()
    maskT = nc.dram_tensor("maskT", [128, 896], F32, kind="ExternalInput").ap()
    permT = nc.dram_tensor("permT", [128, 128], BF, kind="ExternalInput").ap()
    selrowT = nc.dram_tensor("selrowT", [2, 256], BF, kind="ExternalInput").ap()
    # zout stored in SBUF-staging order: [step*4 + oc0//4, p, t, r] =
    # z[(oc0+t)*128 + p, step*PB + r] — 4KB contiguous per partition per DMA
    zout = nc.dram_tensor("zout", [(BT // PB) * 4, 128, 4, PB], BF,
                          kind="ExternalOutput").ap()

    with tile.TileContext(nc) as tc:
        with tc.tile_pool(name="sb", bufs=1) as sb, \
             tc.tile_pool(name="ps", bufs=1, space="PSUM") as ps:
            # ---- x panel tiles: single DMA per panel, 4-deep ring ----
            xt_tiles = {}

            def issue_x(g, eng, split=False):
                xt = sb.tile([128, CC, PB], BF, tag="xt", bufs=4,
                             name=f"xt_{g}")
                src = xtiles[g].rearrange("p (cc r) -> p cc r", r=PB)
                if split:
                    HC = CC // 2
                    nc.sync.dma_start(out=xt[:, :HC, :], in_=src[:, :HC, :])
                    nc.scalar.dma_start(out=xt[:, HC:, :], in_=src[:, HC:, :])
                else:
                    eng.dma_start(out=xt[:], in_=src)
                xt_tiles[g] = xt

            # ---- startup: spread the gating loads over all three queues,
            # ordered by first use ----
            wqkv_sb = sb.tile([128, FT, CC, 128], BF, tag="wqkv", bufs=1)
            wsrc = wqkvT.rearrange("p (ft cc m) -> p ft cc m", ft=FT, cc=CC)
            HC = CC // 2
            xt0 = sb.tile([128, CC, PB], BF, tag="xt", bufs=4, name="xt_0")
            src0 = xtiles[0].rearrange("p (cc r) -> p cc r", r=PB)
            nc.sync.dma_start(out=xt0[:, :HC, :], in_=src0[:, :HC, :])
            nc.gpsimd.dma_start(out=xt0[:, HC:, :], in_=src0[:, HC:, :])
            xt_tiles[0] = xt0
            nc.scalar.dma_start(out=wqkv_sb[:, 2 * HPC:2 * HPC + 1],
                                in_=wsrc[:, 2 * HPC:2 * HPC + 1])
            nc.scalar.dma_start(out=wqkv_sb[:, 2 * HPC + 1:],
                                in_=wsrc[:, 2 * HPC + 1:])
            nc.sync.dma_start(out=wqkv_sb[:, 0:1], in_=wsrc[:, 0:1])
            perm_sb = sb.tile([128, 128], BF, tag="perm", bufs=1)
            nc.sync.dma_start(out=perm_sb[:], in_=permT)
            cos_sb = sb.tile([128, T], BF, tag="cos", bufs=1)
            nc.scalar.dma_start(out=cos_sb[:], in_=cosT)
            sin_sb = sb.tile([128, T], BF, tag="sin", bufs=1)
            nc.scalar.dma_start(out=sin_sb[:], in_=sinT)
            nc.sync.dma_start(out=wqkv_sb[:, 1:HPC + 1],
                              in_=wsrc[:, 1:HPC + 1])
            nc.scalar.dma_start(out=wqkv_sb[:, HPC + 1:2 * HPC],
                                in_=wsrc[:, HPC + 1:2 * HPC])
            if NG > 1:
                issue_x(1, nc.gpsimd)
            mask_sb = sb.tile([128, 896], F32, tag="mask", bufs=1)
            nc.gpsimd.dma_start(out=mask_sb[:], in_=maskT)
            selrow = sb.tile([2, 256], BF, tag="selrow", bufs=1)
            nc.gpsimd.dma_start(out=selrow[:], in_=selrowT)
            wproj_sb = sb.tile([128, HPC, C], BF, tag="wproj", bufs=1)
            nc.scalar.dma_start(
                out=wproj_sb[:],
                in_=wprojT.rearrange("p (hh o) -> p hh o", o=C))
            if NG > 2:
                issue_x(2, nc.sync)
            if NG > 3:
                issue_x(3, nc.scalar)
            # selector columns: sel[:, 2h:2h+2] puts head h's denominator
            # in row h of the shared [2, PB] denominator bank
            sel = sb.tile([128, 4], BF, tag="sel", bufs=1)
            nc.vector.memset(sel[:, 0:1], 1.0)
            nc.vector.memset(sel[:, 1:2], 0.0)
            nc.vector.memset(sel[:, 2:3], 0.0)
            nc.vector.memset(sel[:, 3:4], 1.0)

            # ---- per-batch persistent tiles ----
            qkv_state = {}

            def get_state(b):
                if b not in qkv_state:
                    qkv_state[b] = {
                        "q": [sb.tile([128, T], BF, tag=f"q{h}", bufs=2,
                                      name=f"q{h}_{b}") for h in range(HPC)],
                        "k": [sb.tile([128, T], BF, tag=f"k{h}", bufs=2,
                                      name=f"k{h}_{b}") for h in range(HPC)],
                        "v": [sb.tile([128, T // 128, 128], BF, tag=f"v{h}",
                                      bufs=2, name=f"v{h}_{b}")
                              for h in range(HPC)],
                    }
                return qkv_state[b]

            yp_store = {}
            holders = {}

            # ================= stream builders =================
            def proj_units(g):
                """QKV projection + rope for panel g: list of (cost, fn)."""
                b, pp = divmod(g, NPB)
                st = get_state(b)
                ts = slice(pp * PB, pp * PB + PB)
                xt = xt_tiles[g]
                units = []
                for ft in list(range(2 * HPC, FT)) + list(range(2 * HPC)):
                    def chunk(ft=ft, c0=0):
                        if c0 == 0:
                            holders[(g, ft)] = ps.tile([128, PB], F32,
                                                       tag="pps", bufs=2,
                                                       name=f"pps_{g}_{ft}")
                        pps = holders[(g, ft)]
                        for cc in range(c0, c0 + 4):
                            nc.tensor.matmul(
                                pps[:],
                                lhsT=wqkv_sb[:, ft, cc, :],
                                rhs=xt[:, cc, :],
                                start=(cc == 0), stop=(cc == CC - 1))

                    for c0 in range(0, CC, 4):
                        units.append((4, lambda ft=ft, c0=c0: chunk(ft, c0)))

                    if ft < 2 * HPC:   # q or k: apply rope
                        def rope_ep(ft=ft):
                            pps = holders.pop((g, ft))
                            raw = sb.tile([128, PB], BF, tag="qkraw", bufs=2)
                            nc.scalar.copy(out=raw[:], in_=pps[:])
                            rot = ps.tile([128, PB], F32, tag="pps", bufs=2,
                                          name=f"rot_{g}_{ft}")
                            nc.tensor.matmul(rot[:], lhsT=perm_sb[:],
                                             rhs=raw[:], start=True, stop=True)
                            t1 = sb.tile([128, PB], F32, tag="t1", bufs=2)
                            nc.vector.tensor_mul(out=t1[:], in0=raw[:],
                                                 in1=cos_sb[:, ts])
                            t2 = sb.tile([128, PB], F32, tag="t2", bufs=2)
                            nc.vector.tensor_mul(out=t2[:], in0=rot[:],
                                                 in1=sin_sb[:, ts])
                            dest = (st["q"] if ft < HPC else st["k"])[ft % HPC]
                            nc.vector.tensor_add(out=dest[:, ts], in0=t1[:],
                                                 in1=t2[:])
                        units.append((2, rope_ep))
                    else:              # v: stage + dma-transpose (sync queue)
                        def v_ep(ft=ft):
                            pps = holders.pop((g, ft))
                            h = ft - 2 * HPC
                            vst = sb.tile([128, PB], BF, tag="vstage", bufs=2)
                            nc.scalar.copy(out=vst[:], in_=pps[:])
                            for q4 in range(PB // 128):
                                jt = pp * (PB // 128) + q4
                                nc.sync.dma_start_transpose(
                                    out=st["v"][h][:, jt, :],
                                    in_=vst[:, q4 * 128:(q4 + 1) * 128])
                        units.append((2, v_ep))
                return units

            def attn_units(a):
                """Attention for panel a: list of (cost, fn)."""
                b, pp = divmod(a, NPB)
                st = get_state(b)
                nj = (pp + 1) * NJP
                q0 = pp * PB
                ytil = [ps.tile([128, PB], F32, tag="ytil", bufs=2,
                                name=f"ytil{h}_{a}") for h in range(HPC)]
                den = ps.tile([2, PB], F32, tag="den", bufs=1,
                              name=f"den_{a}")
                # process key-blocks in PAIRS, grouped by head, so the
                # den/PV accumulates can be batched back-to-back into the
                # same PSUM bank (accumulating MMs pay ~+94ns after a
                # bank switch): PV batched over 2 same-head pairs, den
                # over 4 pairs (nj is always a multiple of 4)
                pairs = [(h, j0) for h in range(HPC)
                         for j0 in range(0, nj, 2)]
                spss = {}

                def lof(j):
                    return max(j - pp * NJP, 0) * 128

                def emit_S(h, j):
                    lo = lof(j)
                    sps = ps.tile([128, PB], F32, tag="tr", bufs=3,
                                  name=f"s{h}_{a}_{j}")
                    nc.tensor.matmul(
                        sps[:, lo:PB],
                        lhsT=st["k"][h][:, j * JB:(j + 1) * JB],
                        rhs=st["q"][h][:, q0 + lo:q0 + PB],
                        start=True, stop=True)
                    spss[(h, j)] = sps

                def emit_exp(h, j):
                    kk = j - pp * NJP
                    lo = lof(j)
                    sps = spss.pop((h, j))
                    e = sb.tile([128, PB], BF, tag="e", bufs=8,
                                name=f"e{h}_{a}_{j}")
                    if kk >= 0:
                        nc.vector.scalar_tensor_tensor(
                            out=sps[:, lo:PB],
                            in0=sps[:, lo:PB], scalar=scale,
                            in1=mask_sb[:, 384:896 - lo],
                            op0=mybir.AluOpType.mult,
                            op1=mybir.AluOpType.add)
                        nc.scalar.activation(
                            out=e[:, lo:PB], in_=sps[:, lo:PB],
                            func=mybir.ActivationFunctionType.Exp)
                    else:
                        nc.scalar.activation(
                            out=e[:, lo:PB], in_=sps[:, lo:PB],
                            func=mybir.ActivationFunctionType.Exp,
                            scale=scale)
                    return e

                units = []

                def pre_unit():
                    emit_S(pairs[0][0], pairs[0][1])
                    emit_S(pairs[0][0], pairs[0][1] + 1)
                units.append((2, pre_unit))

                epair = {}
                NPH = nj // 2      # pairs per head

                for k in range(len(pairs)):
                    def unit(k=k):
                        h, j0 = pairs[k]
                        loc = k % NPH
                        es = [emit_exp(h, j0), emit_exp(h, j0 + 1)]
                        epair[k] = es
                        if k + 1 < len(pairs):
                            hn, jn = pairs[k + 1]
                            emit_S(hn, jn)
                            emit_S(hn, jn + 1)
                        if loc % 2 == 1:
                            # PV batched over 2 same-head pairs: 4
                            # back-to-back accumulates into ytil[h]
                            for kk2 in (k - 1, k):
                                jj0 = pairs[kk2][1]
                                for t, j in enumerate((jj0, jj0 + 1)):
                                    lo = lof(j)
                                    nc.tensor.matmul(
                                        ytil[h][:, lo:PB],
                                        lhsT=st["v"][h][:, j, :],
                                        rhs=epair[kk2][t][:, lo:PB],
                                        start=(j == 0), stop=(j == nj - 1))
                        if k % 4 == 3:
                            # denominators batched over the last 4 pairs:
                            # 8 back-to-back accumulates into the den bank
                            for kk2 in range(k - 3, k + 1):
                                hh, jj0 = pairs[kk2]
                                for t, j in enumerate((jj0, jj0 + 1)):
                                    lo = lof(j)
                                    nc.tensor.matmul(
                                        den[:, lo:PB],
                                        lhsT=sel[:, 2 * hh:2 * hh + 2],
                                        rhs=epair[kk2][t][:, lo:PB],
                                        start=(j == 0 and hh == 0),
                                        stop=(j == nj - 1 and hh == HPC - 1))
                                if kk2 <= k - 2:
                                    del epair[kk2]
                            del epair[k - 1], epair[k]
                    units.append((6, unit))

                def normalize():
                    dbf = sb.tile([2, PB], BF, tag="dbf", bufs=2)
                    nc.scalar.copy(out=dbf[:], in_=den[:])
                    ypair = []
                    for h in range(HPC):
                        bc = ps.tile([128, PB], F32, tag="tr", bufs=3,
                                     name=f"bc{h}_{a}")
                        nc.tensor.matmul(bc[:],
                                         lhsT=selrow[:, h * 128:(h + 1) * 128],
                                         rhs=dbf[:], start=True, stop=True)
                        rec = sb.tile([128, PB], F32, tag="rec", bufs=2)
                        nc.vector.reciprocal_approx_fast(out=rec[:], in_=bc[:])
                        yp = sb.tile([128, PB], BF, tag="yp", bufs=6)
                        nc.vector.tensor_mul(out=yp[:], in0=ytil[h][:],
                                             in1=rec[:])
                        ypair.append(yp)
                    yp_store[a] = ypair
                units.append((3, normalize))
                return units

            def outproj_units(o):
                """Output projection for panel o: list of (cost, fn)."""
                ypair = yp_store.pop(o)
                units = []
                last = (o == NG - 1)   # finer DMA granularity for the drain
                for oc0 in range(0, NOC, 4):
                    def mm_unit(oc0=oc0, t0=0):
                        if t0 == 0:
                            holders[("z", o)] = sb.tile([128, 4, PB], BF,
                                                        tag="zstg", bufs=3,
                                                        name=f"zstg_{o}_{oc0}")
                        zstg = holders[("z", o)]
                        for t in range(t0, t0 + 2):
                            oc = oc0 + t
                            zps = ps.tile([128, PB], F32, tag="tr", bufs=3,
                                          name=f"z_{o}_{oc}")
                            for hh in range(HPC):
                                nc.tensor.matmul(
                                    zps[:],
                                    lhsT=wproj_sb[:, hh, oc * 128:(oc + 1) * 128],
                                    rhs=ypair[hh][:],
                                    start=(hh == 0), stop=(hh == HPC - 1))
                            nc.vector.tensor_copy(out=zstg[:, t, :], in_=zps[:])
                        if last:
                            eng = nc.gpsimd if (oc0 + t0) % 8 < 4 else nc.scalar
                            eng.dma_start(
                                out=zout[o * 4 + oc0 // 4][:, t0:t0 + 2, :],
                                in_=zstg[:, t0:t0 + 2, :])
                        elif t0 == 2:
                            nc.gpsimd.dma_start(
                                out=zout[o * 4 + oc0 // 4], in_=zstg[:])
                        if t0 == 2:
                            del holders[("z", o)]
                    units.append((5, lambda oc0=oc0: mm_unit(oc0, 0)))
                    units.append((5, lambda oc0=oc0: mm_unit(oc0, 2)))
                return units

            # ================= weave & emit =================
            def weave(lists):
                streams = []
                for units in lists:
                    if not units:
                        continue
                    total = float(sum(c for c, _ in units))
                    acc = 0.0
                    for c, fn in units:
                        streams.append(((acc + 0.5 * c) / total, fn))
                        acc += c
                streams.sort(key=lambda x: x[0])
                for _, fn in streams:
                    fn()

            for step in range(NG + 2):
                lists = []
                if step < NG:
                    lists.append(proj_units(step))
                if 0 <= step - 1 < NG:
                    lists.append(attn_units(step - 1))
                if 0 <= step - 2 < NG:
                    lists.append(outproj_units(step - 2))
                weave(lists)
                if step + 4 < NG:
                    issue_x(step + 4,
                            nc.scalar if (step + 4) % 2 == 0 else nc.sync)

    nc.compile()
    return nc


_module_cache = {}


def _get_module(B, T):
    key = (B, T)
    if key not in _module_cache:
        _module_cache[key] = build_module(B, T)
    return _module_cache[key]


def _host_prep(x, Wqkv, Wproj, B, T):
    bf16 = ml_dtypes.bfloat16
    BT = B * T
    NP = BT // PB
    CC = C // 128
    FT = 3 * HPC
    x2 = x.reshape(NP, PB, CC, 128)
    xtiles = np.ascontiguousarray(
        x2.transpose(0, 3, 2, 1).reshape(NP, 128, CC * PB)).astype(bf16)

    inv = 1.0 / (ROPE_BASE ** (np.arange(0, D, 2, dtype=np.float32) / D))
    t = np.arange(T, dtype=np.float32)
    fr = np.outer(t, inv)                      # [T, 64]
    emb = np.concatenate([fr, fr], -1)         # [T, 128]
    cosT = np.ascontiguousarray(np.cos(emb).T).astype(bf16)
    sinT = np.ascontiguousarray(np.sin(emb).T).astype(bf16)

    g = np.arange(896)[None, :]
    p = np.arange(128)[:, None]
    maskT = np.where(g >= p + 384, 0.0, NEG).astype(np.float32)

    permT = np.zeros((128, 128), np.float32)
    for j in range(64):
        permT[j, j + 64] = 1.0                 # rot[i] = q[i-64] for i>=64
    for j in range(64, 128):
        permT[j, j - 64] = -1.0                # rot[i] = -q[i+64] for i<64
    permT = permT.astype(bf16)

    selrowT = np.zeros((2, 256), np.float32)
    selrowT[0, 0:128] = 1.0
    selrowT[1, 128:256] = 1.0
    selrowT = selrowT.astype(bf16)

    in_maps = []
    for c in range(N_CORES):
        heads = [HPC * c + h for h in range(HPC)]
        rows = []
        for blk in range(3):                   # q, k, v blocks of Wqkv
            for h in heads:
                r0 = blk * C + h * D
                rows.append(Wqkv[r0:r0 + D])
        wslice = np.concatenate(rows, 0)       # [FT*128, C]
        wqkvT = np.ascontiguousarray(           # [128, FT*CC*128] ft-major
            wslice.T.reshape(CC, 128, FT, 128).transpose(1, 2, 0, 3)
            .reshape(128, FT * CC * 128)).astype(bf16)
        cols = np.concatenate([np.arange(h * D, (h + 1) * D) for h in heads])
        wprojT = np.ascontiguousarray(          # [128, HPC*C] single-run
            Wproj[:, cols].T.reshape(HPC, 128, C).transpose(1, 0, 2)
            .reshape(128, HPC * C)).astype(bf16)
        in_maps.append({
            "xtiles": xtiles,
            "wqkvT": wqkvT,
            "wprojT": wprojT,
            "cosT": cosT,
            "sinT": sinT,
            "maskT": maskT,
            "permT": permT,
            "selrowT": selrowT,
        })
    return in_maps


last_results = None


def kernel(x, Wqkv, Wproj, _trace=False, _trace_kwargs=None):
    global last_results
    x = np.asarray(x, dtype=np.float32)
    Wqkv = np.asarray(Wqkv, dtype=np.float32)
    Wproj = np.asarray(Wproj, dtype=np.float32)
    B, T, _C = x.shape
    assert _C == C and T % PB == 0

    nc = _get_module(B, T)
    in_maps = _host_prep(x, Wqkv, Wproj, B, T)
    res = run_bass_kernel_spmd(nc, in_maps, core_ids=list(range(N_CORES)),
                               trace=_trace, **(_trace_kwargs or {}))
    last_results = res
    z = res.results[0]["zout"].astype(np.float32)
    for c in range(1, N_CORES):
        z += res.results[c]["zout"].astype(np.float32)
    # z: [NG*4, 128, 4, PB] with [o*4+q, p, t, r] = zfull[(4q+t)*128+p, o*PB+r]
    NG = B * T // PB
    z = z.reshape(NG, 4, 128, 4, PB).transpose(1, 3, 2, 0, 4).reshape(C, B * T)
    y = np.ascontiguousarray(z.T).reshape(B, T, C)
    return y


# revision 48
# speedup vs baseline: 1.0202x; 1.0202x over previous
"""Causal self-attention (QKV proj + RoPE + causal SDPA + out proj) on 8 trn2 cores.

Sharding: tensor-parallel over heads. Each core owns 2 of 16 heads:
  - Wqkv column-split (the core's q/k/v head rows), Wproj row-split.
  - Each core computes a full-shape partial of the output projection;
    the 8 partials are summed (and transposed back) on the host.

Device-side layout trick: everything runs transposed. The host feeds
x^T [C, B*T]; the QKV projection computes qkv^T = Wslice @ x with the
head dim on partitions, which is exactly what Q@K^T and the output
projection want as inputs, so no on-chip transposes are needed except
V (done with DMA xbar transposes, off the critical engines).

Scheduling: panel-level software pipeline.  For global panel g the
emitter weaves three independent instruction streams into the engine
queues so the PE never starves:
  - QKV projection + rope for panel g      (gated on the x DMA)
  - attention for panel g-1                (gated on ACT exp latency)
  - output projection for panel g-2
x panels arrive via single 2MB DMAs on the sync queue, prefetched 3
panels ahead; V transposes are issued from the sync engine; zout
stores go through the gpsimd SWDGE two output-tiles at a time.
"""
import sys

sys.path.insert(0, "/opt/trn_rl_repo")

import numpy as np
import ml_dtypes

import concourse.bacc as bacc
import concourse.mybir as mybir
import concourse.tile as tile
from concourse.bass_utils import run_bass_kernel_spmd

N_CORES = 8
C = 2048
H = 16
D = 128
HPC = H // N_CORES          # heads per core = 2
PB = 512                    # row panel width
JB = 128                    # key tile width
NEG = -1.0e30
ROPE_BASE = 10000.0

BF = mybir.dt.bfloat16
F32 = mybir.dt.float32


def build_module(B, T):
    BT = B * T
    CC = C // 128            # contraction chunks for the projection
    FT = 3 * HPC             # qkv f-tiles per core (q0 q1 k0 k1 v0 v1)
    NPB = T // PB            # panels per batch
    NG = B * NPB             # global panels
    NOC = C // 128           # out-proj column tiles
    NJP = PB // JB           # key blocks per panel
    scale = 1.0 / float(np.sqrt(D))

    nc = bacc.Bacc("TRN2", target_bir_lowering=False, debug=False,
                   num_devices=N_CORES)

    # x pre-tiled on host: xtiles[g, p, cc*PB + r] = x[g*PB + r, cc*128 + p]
    xtiles = nc.dram_tensor("xtiles", [NG, 128, CC * PB], BF,
                            kind="ExternalInput").ap()
    # weights pre-arranged on host so each partition is one contiguous run
    wqkvT = nc.dram_tensor("wqkvT", [128, CC * FT * 128], BF,
                           kind="ExternalInput").ap()
    wprojT = nc.dram_tensor("wprojT", [128, HPC * C], BF,
                            kind="ExternalInput").ap()
    cosT = nc.dram_tensor("cosT", [128, T], BF, kind="ExternalInput").ap()
    sinT = nc.dram_tensor("sinT", [128, T], BF, kind="ExternalInput").ap()
    maskT = nc.dram_tensor("maskT", [128, 896], F32, kind="ExternalInput").ap()
    permT = nc.dram_tensor("permT", [128, 128], BF, kind="ExternalInput").ap()
    selrowT = nc.dram_tensor("selrowT", [2, 256], BF, kind="ExternalInput").ap()
    # zout stored in SBUF-staging order: [step*4 + oc0//4, p, t, r] =
    # z[(oc0+t)*128 + p, step*PB + r] — 4KB contiguous per partition per DMA
    zout = nc.dram_tensor("zout", [(BT // PB) * 4, 128, 4, PB], BF,
                          kind="ExternalOutput").ap()

    with tile.TileContext(nc) as tc:
        with tc.tile_pool(name="sb", bufs=1) as sb, \
             tc.tile_pool(name="ps", bufs=1, space="PSUM") as ps:
            # ---- x panel tiles: single DMA per panel, 4-deep ring ----
            xt_tiles = {}

            def issue_x(g, eng, split=False):
                xt = sb.tile([128, CC, PB], BF, tag="xt", bufs=4,
                             name=f"xt_{g}")
                src = xtiles[g].rearrange("p (cc r) -> p cc r", r=PB)
                if split:
                    HC = CC // 2
                    nc.sync.dma_start(out=xt[:, :HC, :], in_=src[:, :HC, :])
                    nc.scalar.dma_start(out=xt[:, HC:, :], in_=src[:, HC:, :])
                else:
                    eng.dma_start(out=xt[:], in_=src)
                xt_tiles[g] = xt

            # ---- startup: spread the gating loads over all three queues,
            # ordered by first use ----
            wqkv_sb = sb.tile([128, FT, CC, 128], BF, tag="wqkv", bufs=1)
            wsrc = wqkvT.rearrange("p (ft cc m) -> p ft cc m", ft=FT, cc=CC)
            HC = CC // 2
            xt0 = sb.tile([128, CC, PB], BF, tag="xt", bufs=4, name="xt_0")
            src0 = xtiles[0].rearrange("p (cc r) -> p cc r", r=PB)
            nc.sync.dma_start(out=xt0[:, :HC, :], in_=src0[:, :HC, :])
            nc.gpsimd.dma_start(out=xt0[:, HC:, :], in_=src0[:, HC:, :])
            xt_tiles[0] = xt0
            nc.scalar.dma_start(out=wqkv_sb[:, 2 * HPC:2 * HPC + 1],
                                in_=wsrc[:, 2 * HPC:2 * HPC + 1])
            nc.scalar.dma_start(out=wqkv_sb[:, 2 * HPC + 1:],
                                in_=wsrc[:, 2 * HPC + 1:])
            nc.sync.dma_start(out=wqkv_sb[:, 0:1], in_=wsrc[:, 0:1])
            perm_sb = sb.tile([128, 128], BF, tag="perm", bufs=1)
            nc.sync.dma_start(out=perm_sb[:], in_=permT)
            cos_sb = sb.tile([128, T], BF, tag="cos", bufs=1)
            nc.scalar.dma_start(out=cos_sb[:], in_=cosT)
            sin_sb = sb.tile([128, T], BF, tag="sin", bufs=1)
            nc.scalar.dma_start(out=sin_sb[:], in_=sinT)
            nc.sync.dma_start(out=wqkv_sb[:, 1:HPC + 1],
                              in_=wsrc[:, 1:HPC + 1])
            nc.scalar.dma_start(out=wqkv_sb[:, HPC + 1:2 * HPC],
                                in_=wsrc[:, HPC + 1:2 * HPC])
            if NG > 1:
                issue_x(1, nc.gpsimd)
            mask_sb = sb.tile([128, 896], F32, tag="mask", bufs=1)
            nc.gpsimd.dma_start(out=mask_sb[:], in_=maskT)
            selrow = sb.tile([2, 256], BF, tag="selrow", bufs=1)
            nc.gpsimd.dma_start(out=selrow[:], in_=selrowT)
            wproj_sb = sb.tile([128, HPC, C], BF, tag="wproj", bufs=1)
            nc.scalar.dma_start(
                out=wproj_sb[:],
                in_=wprojT.rearrange("p (hh o) -> p hh o", o=C))
            if NG > 2:
                issue_x(2, nc.sync)
            # selector columns: sel[:, 2h:2h+2] puts head h's denominator
            # in row h of the shared [2, PB] denominator bank
            sel = sb.tile([128, 4], BF, tag="sel", bufs=1)
            nc.vector.memset(sel[:, 0:1], 1.0)
            nc.vector.memset(sel[:, 1:2], 0.0)
            nc.vector.memset(sel[:, 2:3], 0.0)
            nc.vector.memset(sel[:, 3:4], 1.0)

            # ---- per-batch persistent tiles ----
            qkv_state = {}

            def get_state(b):
                if b not in qkv_state:
                    qkv_state[b] = {
                        "q": [sb.tile([128, T], BF, tag=f"q{h}", bufs=2,
                                      name=f"q{h}_{b}") for h in range(HPC)],
                        "k": [sb.tile([128, T], BF, tag=f"k{h}", bufs=2,
                                      name=f"k{h}_{b}") for h in range(HPC)],
                        "v": [sb.tile([128, T // 128, 128], BF, tag=f"v{h}",
                                      bufs=2, name=f"v{h}_{b}")
                              for h in range(HPC)],
                    }
                return qkv_state[b]

            yp_store = {}
            holders = {}

            # ================= stream builders =================
            def proj_units(g):
                """QKV projection + rope for panel g: list of (cost, fn)."""
                b, pp = divmod(g, NPB)
                st = get_state(b)
                ts = slice(pp * PB, pp * PB + PB)
                xt = xt_tiles[g]
                units = []
                for ft in list(range(2 * HPC, FT)) + list(range(2 * HPC)):
                    def chunk(ft=ft, c0=0):
                        if c0 == 0:
                            holders[(g, ft)] = ps.tile([128, PB], F32,
                                                       tag="pps", bufs=2,
                                                       name=f"pps_{g}_{ft}")
                        pps = holders[(g, ft)]
                        for cc in range(c0, c0 + 4):
                            nc.tensor.matmul(
                                pps[:],
                                lhsT=wqkv_sb[:, ft, cc, :],
                                rhs=xt[:, cc, :],
                                start=(cc == 0), stop=(cc == CC - 1))

                    for c0 in range(0, CC, 4):
                        units.append((4, lambda ft=ft, c0=c0: chunk(ft, c0)))

                    if ft < 2 * HPC:   # q or k: apply rope
                        def rope_ep(ft=ft):
                            pps = holders.pop((g, ft))
                            raw = sb.tile([128, PB], BF, tag="qkraw", bufs=2)
                            nc.scalar.copy(out=raw[:], in_=pps[:])
                            rot = ps.tile([128, PB], F32, tag="pps", bufs=2,
                                          name=f"rot_{g}_{ft}")
                            nc.tensor.matmul(rot[:], lhsT=perm_sb[:],
                                             rhs=raw[:], start=True, stop=True)
                            t1 = sb.tile([128, PB], F32, tag="t1", bufs=2)
                            nc.vector.tensor_mul(out=t1[:], in0=raw[:],
                                                 in1=cos_sb[:, ts])
                            t2 = sb.tile([128, PB], F32, tag="t2", bufs=2)
                            nc.vector.tensor_mul(out=t2[:], in0=rot[:],
                                                 in1=sin_sb[:, ts])
                            dest = (st["q"] if ft < HPC else st["k"])[ft % HPC]
                            nc.vector.tensor_add(out=dest[:, ts], in0=t1[:],
                                                 in1=t2[:])
                        units.append((2, rope_ep))
                    else:              # v: stage + dma-transpose (sync queue)
                        def v_ep(ft=ft):
                            pps = holders.pop((g, ft))
                            h = ft - 2 * HPC
                            vst = sb.tile([128, PB], BF, tag="vstage", bufs=2)
                            nc.scalar.copy(out=vst[:], in_=pps[:])
                            for q4 in range(PB // 128):
                                jt = pp * (PB // 128) + q4
                                nc.sync.dma_start_transpose(
                                    out=st["v"][h][:, jt, :],
                                    in_=vst[:, q4 * 128:(q4 + 1) * 128])
                        units.append((2, v_ep))
                return units

            def attn_units(a):
                """Attention for panel a: list of (cost, fn)."""
                b, pp = divmod(a, NPB)
                st = get_state(b)
                nj = (pp + 1) * NJP
                q0 = pp * PB
                ytil = [ps.tile([128, PB], F32, tag="ytil", bufs=2,
                                name=f"ytil{h}_{a}") for h in range(HPC)]
                den = ps.tile([2, PB], F32, tag="den", bufs=1,
                              name=f"den_{a}")
                # process key-blocks in PAIRS, grouped by head, so the
                # den/PV accumulates can be batched back-to-back into the
                # same PSUM bank (accumulating MMs pay ~+94ns after a
                # bank switch): PV batched over 2 same-head pairs, den
                # over 4 pairs (nj is always a multiple of 4)
                pairs = [(h, j0) for h in range(HPC)
                         for j0 in range(0, nj, 2)]
                spss = {}

                def lof(j):
                    return max(j - pp * NJP, 0) * 128

                def emit_S(h, j):
                    lo = lof(j)
                    sps = ps.tile([128, PB], F32, tag="tr", bufs=3,
                                  name=f"s{h}_{a}_{j}")
                    nc.tensor.matmul(
                        sps[:, lo:PB],
                        lhsT=st["k"][h][:, j * JB:(j + 1) * JB],
                        rhs=st["q"][h][:, q0 + lo:q0 + PB],
                        start=True, stop=True)
                    spss[(h, j)] = sps

                def emit_exp(h, j):
                    kk = j - pp * NJP
                    lo = lof(j)
                    sps = spss.pop((h, j))
                    e = sb.tile([128, PB], BF, tag="e", bufs=8,
                                name=f"e{h}_{a}_{j}")
                    if kk >= 0:
                        nc.vector.scalar_tensor_tensor(
                            out=sps[:, lo:PB],
                            in0=sps[:, lo:PB], scalar=scale,
                            in1=mask_sb[:, 384:896 - lo],
                            op0=mybir.AluOpType.mult,
                            op1=mybir.AluOpType.add)
                        nc.scalar.activation(
                            out=e[:, lo:PB], in_=sps[:, lo:PB],
                            func=mybir.ActivationFunctionType.Exp)
                    else:
                        nc.scalar.activation(
                            out=e[:, lo:PB], in_=sps[:, lo:PB],
                            func=mybir.ActivationFunctionType.Exp,
                            scale=scale)
                    return e

                units = []

                def pre_unit():
                    emit_S(pairs[0][0], pairs[0][1])
                    emit_S(pairs[0][0], pairs[0][1] + 1)
                units.append((2, pre_unit))

                epair = {}
                pvpend = []
                NPH = nj // 2      # pairs per head

                for k in range(len(pairs)):
                    def unit(k=k):
                        h, j0 = pairs[k]
                        loc = k % NPH
                        es = [emit_exp(h, j0), emit_exp(h, j0 + 1)]
                        epair[k] = es
                        pvpend.append(k)
                        if k + 1 < len(pairs):
                            hn, jn = pairs[k + 1]
                            emit_S(hn, jn)
                            emit_S(hn, jn + 1)
                        if loc % 4 == 3 or loc == NPH - 1:
                            # PV batched over up to 4 same-head pairs:
                            # 8 back-to-back accumulates into ytil[h]
                            for kk2 in pvpend:
                                jj0 = pairs[kk2][1]
                                for t, j in enumerate((jj0, jj0 + 1)):
                                    lo = lof(j)
                                    nc.tensor.matmul(
                                        ytil[h][:, lo:PB],
                                        lhsT=st["v"][h][:, j, :],
                                        rhs=epair[kk2][t][:, lo:PB],
                                        start=(j == 0), stop=(j == nj - 1))
                            pvpend.clear()
                        if k % 4 == 3:
                            # denominators batched over the last 4 pairs:
                            # 8 back-to-back accumulates into the den bank
                            for kk2 in range(k - 3, k + 1):
                                hh, jj0 = pairs[kk2]
                                for t, j in enumerate((jj0, jj0 + 1)):
                                    lo = lof(j)
                                    nc.tensor.matmul(
                                        den[:, lo:PB],
                                        lhsT=sel[:, 2 * hh:2 * hh + 2],
                                        rhs=epair[kk2][t][:, lo:PB],
                                        start=(j == 0 and hh == 0),
                                        stop=(j == nj - 1 and hh == HPC - 1))
                    units.append((6, unit))

                def normalize():
                    dbf = sb.tile([2, PB], BF, tag="dbf", bufs=2)
                    nc.scalar.copy(out=dbf[:], in_=den[:])
                    ypair = []
                    for h in range(HPC):
                        bc = ps.tile([128, PB], F32, tag="tr", bufs=3,
                                     name=f"bc{h}_{a}")
                        nc.tensor.matmul(bc[:],
                                         lhsT=selrow[:, h * 128:(h + 1) * 128],
                                         rhs=dbf[:], start=True, stop=True)
                        rec = sb.tile([128, PB], F32, tag="rec", bufs=2)
                        nc.vector.reciprocal_approx_fast(out=rec[:], in_=bc[:])
                        yp = sb.tile([128, PB], BF, tag="yp", bufs=6)
                        nc.vector.tensor_mul(out=yp[:], in0=ytil[h][:],
                                             in1=rec[:])
                        ypair.append(yp)
                    yp_store[a] = ypair
                units.append((3, normalize))
                return units

            def outproj_units(o):
                """Output projection for panel o: list of (cost, fn)."""
                ypair = yp_store.pop(o)
                units = []
                last = (o == NG - 1)   # finer DMA granularity for the drain
                for oc0 in range(0, NOC, 4):
                    def mm_unit(oc0=oc0, t0=0):
                        if t0 == 0:
                            holders[("z", o)] = sb.tile([128, 4, PB], BF,
                                                        tag="zstg", bufs=3,
                                                        name=f"zstg_{o}_{oc0}")
                        zstg = holders[("z", o)]
                        for t in range(t0, t0 + 2):
                            oc = oc0 + t
                            zps = ps.tile([128, PB], F32, tag="tr", bufs=3,
                                          name=f"z_{o}_{oc}")
                            for hh in range(HPC):
                                nc.tensor.matmul(
                                    zps[:],
                                    lhsT=wproj_sb[:, hh, oc * 128:(oc + 1) * 128],
                                    rhs=ypair[hh][:],
                                    start=(hh == 0), stop=(hh == HPC - 1))
                            nc.vector.tensor_copy(out=zstg[:, t, :], in_=zps[:])
                        if last:
                            eng = nc.gpsimd if (oc0 + t0) % 8 < 4 else nc.scalar
                            eng.dma_start(
                                out=zout[o * 4 + oc0 // 4][:, t0:t0 + 2, :],
                                in_=zstg[:, t0:t0 + 2, :])
                        elif t0 == 2:
                            nc.gpsimd.dma_start(
                                out=zout[o * 4 + oc0 // 4], in_=zstg[:])
                        if t0 == 2:
                            del holders[("z", o)]
                    units.append((5, lambda oc0=oc0: mm_unit(oc0, 0)))
                    units.append((5, lambda oc0=oc0: mm_unit(oc0, 2)))
                return units

            # ================= weave & emit =================
            def weave(lists):
                streams = []
                for units in lists:
                    if not units:
                        continue
                    total = float(sum(c for c, _ in units))
                    acc = 0.0
                    for c, fn in units:
                        streams.append(((acc + 0.5 * c) / total, fn))
                        acc += c
                streams.sort(key=lambda x: x[0])
                for _, fn in streams:
                    fn()

            for step in range(NG + 2):
                lists = []
                if step < NG:
                    lists.append(proj_units(step))
                if 0 <= step - 1 < NG:
                    lists.append(attn_units(step - 1))
                if 0 <= step - 2 < NG:
                    lists.append(outproj_units(step - 2))
                weave(lists)
                if step + 3 < NG:
                    issue_x(step + 3,
                            nc.scalar if (step + 3) % 2 == 0 else nc.sync)

    nc.compile()
    return nc


_module_cache = {}


def _get_module(B, T):
    key = (B, T)
    if key not in _module_cache:
        _module_cache[key] = build_module(B, T)
    return _module_cache[key]


def _host_prep(x, Wqkv, Wproj, B, T):
    bf16 = ml_dtypes.bfloat16
    BT = B * T
    NP = BT // PB
    CC = C // 128
    FT = 3 * HPC
    x2 = x.reshape(NP, PB, CC, 128)
    xtiles = np.ascontiguousarray(
        x2.transpose(0, 3, 2, 1).reshape(NP, 128, CC * PB)).astype(bf16)

    inv = 1.0 / (ROPE_BASE ** (np.arange(0, D, 2, dtype=np.float32) / D))
    t = np.arange(T, dtype=np.float32)
    fr = np.outer(t, inv)                      # [T, 64]
    emb = np.concatenate([fr, fr], -1)         # [T, 128]
    cosT = np.ascontiguousarray(np.cos(emb).T).astype(bf16)
    sinT = np.ascontiguousarray(np.sin(emb).T).astype(bf16)

    g = np.arange(896)[None, :]
    p = np.arange(128)[:, None]
    maskT = np.where(g >= p + 384, 0.0, NEG).astype(np.float32)

    permT = np.zeros((128, 128), np.float32)
    for j in range(64):
        permT[j, j + 64] = 1.0                 # rot[i] = q[i-64] for i>=64
    for j in range(64, 128):
        permT[j, j - 64] = -1.0                # rot[i] = -q[i+64] for i<64
    permT = permT.astype(bf16)

    selrowT = np.zeros((2, 256), np.float32)
    selrowT[0, 0:128] = 1.0
    selrowT[1, 128:256] = 1.0
    selrowT = selrowT.astype(bf16)

    in_maps = []
    for c in range(N_CORES):
        heads = [HPC * c + h for h in range(HPC)]
        rows = []
        for blk in range(3):                   # q, k, v blocks of Wqkv
            for h in heads:
                r0 = blk * C + h * D
                rows.append(Wqkv[r0:r0 + D])
        wslice = np.concatenate(rows, 0)       # [FT*128, C]
        wqkvT = np.ascontiguousarray(           # [128, FT*CC*128] ft-major
            wslice.T.reshape(CC, 128, FT, 128).transpose(1, 2, 0, 3)
            .reshape(128, FT * CC * 128)).astype(bf16)
        cols = np.concatenate([np.arange(h * D, (h + 1) * D) for h in heads])
        wprojT = np.ascontiguousarray(          # [128, HPC*C] single-run
            Wproj[:, cols].T.reshape(HPC, 128, C).transpose(1, 0, 2)
            .reshape(128, HPC * C)).astype(bf16)
        in_maps.append({
            "xtiles": xtiles,
            "wqkvT": wqkvT,
            "wprojT": wprojT,
            "cosT": cosT,
            "sinT": sinT,
            "maskT": maskT,
            "permT": permT,
            "selrowT": selrowT,
        })
    return in_maps


last_results = None


def kernel(x, Wqkv, Wproj, _trace=False, _trace_kwargs=None):
    global last_results
    x = np.asarray(x, dtype=np.float32)
    Wqkv = np.asarray(Wqkv, dtype=np.float32)
    Wproj = np.asarray(Wproj, dtype=np.float32)
    B, T, _C = x.shape
    assert _C == C and T % PB == 0

    nc = _get_module(B, T)
    in_maps = _host_prep(x, Wqkv, Wproj, B, T)
    res = run_bass_kernel_spmd(nc, in_maps, core_ids=list(range(N_CORES)),
                               trace=_trace, **(_trace_kwargs or {}))
    last_results = res
    z = res.results[0]["zout"].astype(np.float32)
    for c in range(1, N_CORES):
        z += res.results[c]["zout"].astype(np.float32)
    # z: [NG*4, 128, 4, PB] with [o*4+q, p, t, r] = zfull[(4q+t)*128+p, o*PB+r]
    NG = B * T // PB
    z = z.reshape(NG, 4, 128, 4, PB).transpose(1, 3, 2, 0, 4).reshape(C, B * T)
    y = np.ascontiguousarray(z.T).reshape(B, T, C)
    return y
